# revision 34
# baseline (speedup 1.0000x reference)
"""GraphSAGE (2-layer, mean aggregation) on 8 Trainium2 NeuronCores.

Strategy (v5):
  - Nodes sharded across 8 cores by destination; per-core permutation
    balances per-64-row-block in-edge loads (layer1 / layer2-g1 / layer2-g2).
  - Features gathered as fp8 e4m3 PAIRS (row p = nodes 2p, 2p+1, 256 B);
    pair indices fit int16. dma_gather descriptor generation on the Q7
    cores is the hard throughput wall (~1.8 ns/edge), so every other
    engine's work is kept below it:
      * edges of each (block, stream) are sorted by src parity, so almost
        all 128-edge chunks are parity-pure: ONE bf16 one-hot + ONE fp8
        matmul per chunk; only boundary chunks need the dual form.
      * dst blocks are 64 wide, halving one-hot Vector work.
  - Layer-1 h rows are computed per block pair inside the aggregation
    loop, stored to hsh in fp8. The h exchange is TWO AllGathers: rows of
    blocks 0..49 (triggered mid-layer-1) and the rest. Layer-2 edges are
    split by source half (g1 from hfullA, g2 from hfullB) and aggregated
    in two passes with an SBUF accumulator, so g1 gathers and the second
    collective overlap.
"""

import math
from contextlib import ExitStack

import numpy as np
import ml_dtypes

import concourse.bass as bass
import concourse.bacc as bacc
import concourse.mybir as mybir
import concourse.tile as tile
from concourse import bass_utils

P = 128
BW = 64                                # dst block width
N_NODES = 50000
D_IN = 128
D_HID = 128
D_OUT = 40
N_CORES = 8
ROWS_PER = N_NODES // N_CORES          # 6250
NBLK = math.ceil(ROWS_PER / BW)        # 98
NBLK1 = 50                             # blocks in half 1
H1_ROWS = NBLK1 * BW                   # 3200
H2_ROWS = ROWS_PER - H1_ROWS           # 3050
GRP = 32                               # chunks per dma_gather call
GBUFS = 8
OBUFS = 6
NQ = 4

BF16 = ml_dtypes.bfloat16
FP8 = ml_dtypes.float8_e4m3


def _wrap_idxs(idx_flat):
    n = idx_flat.shape[0]
    assert n % 16 == 0
    w = idx_flat.reshape(n // 16, 16).T.astype(np.int16)
    return np.tile(w, (8, 1))


def _greedy_assign(items, loads, nbins, caps, targets):
    order = np.argsort(-loads.sum(axis=1), kind="stable")
    binloads = np.zeros((nbins, loads.shape[1]))
    cnt = np.zeros(nbins, np.int64)
    bins = [[] for _ in range(nbins)]
    tgt = np.maximum(targets, 1e-9)
    for i in order:
        cost = ((binloads + loads[i][None, :]) / tgt).max(axis=1)
        cost[cnt >= caps] = np.inf
        b = int(np.argmin(cost))
        bins[b].append(items[i])
        binloads[b] += loads[i]
        cnt[b] += 1
    return bins


def preprocess(edge_index):
    src0 = np.asarray(edge_index[0], dtype=np.int64)
    dst0 = np.asarray(edge_index[1], dtype=np.int64)
    deg_in = np.bincount(dst0, minlength=N_NODES)

    # assign nodes to 64-row blocks, evening per-block in-degree
    perm = np.empty(N_NODES, np.int64)
    for k in range(N_CORES):
        base = k * ROWS_PER
        nodes = np.arange(base, base + ROWS_PER)
        loads = deg_in[nodes].astype(np.float64)[:, None]
        caps = np.full(NBLK, BW, np.int64)
        caps[-1] = ROWS_PER - (NBLK - 1) * BW
        targets = caps[:, None] * (loads.sum() / ROWS_PER)
        bins = _greedy_assign(nodes, loads, NBLK, caps, targets)
        off = base
        for b in range(NBLK):
            ids = np.asarray(bins[b], np.int64)
            perm[off : off + ids.shape[0]] = ids
            off += ids.shape[0]

    slot_of = np.empty(N_NODES, np.int64)
    slot_of[perm] = np.arange(N_NODES)
    src = slot_of[src0]
    dst = slot_of[dst0]
    counts = np.bincount(dst, minlength=N_NODES)
    inv_deg = (1.0 / np.maximum(counts, 1)).astype(np.float32)

    # one shared stream: pair row = src // 2 works for both the x pair
    # table and the AllGather output (6250 even -> core-major pair rows)
    order = np.argsort(dst, kind="stable")
    d_s = dst[order]
    par_s = (src % 2)[order]
    row_s = (src // 2)[order]

    seg = {}
    counts_b = np.zeros(NBLK, np.int64)
    ne = np.zeros((N_CORES, NBLK), np.int64)
    for k in range(N_CORES):
        base = k * ROWS_PER
        for b in range(NBLK):
            r0 = base + b * BW
            r1 = min(base + ROWS_PER, r0 + BW)
            e0 = np.searchsorted(d_s, r0, side="left")
            e1 = np.searchsorted(d_s, r1, side="left")
            sl = slice(e0, e1)
            rr = row_s[sl]
            dd = d_s[sl] - r0
            ev = par_s[sl] == 0
            rr = np.concatenate([rr[ev], rr[~ev]])
            dd = np.concatenate([dd[ev], dd[~ev]])
            seg[(k, b)] = (rr, dd, int(ev.sum()))
            ne[k, b] = int(ev.sum())
            counts_b[b] = max(counts_b[b], (rr.shape[0] + P - 1) // P)

    off_b = np.zeros(NBLK + 1, np.int64)
    off_b[1:] = np.cumsum(counts_b)
    C = int(off_b[-1])
    labels = np.zeros(C, np.int8)
    for b in range(NBLK):
        tmin = int(ne[:, b].min())
        tmax = int(ne[:, b].max())
        for c in range(int(counts_b[b])):
            s0, s1 = c * P, (c + 1) * P
            if s1 <= tmin:
                labels[off_b[b] + c] = 0
            elif s0 >= tmax:
                labels[off_b[b] + c] = 1
            else:
                labels[off_b[b] + c] = 2

    per_core = []
    for k in range(N_CORES):
        idx = np.zeros((C, P), np.int16)
        dstv = np.full((C, P, 2), -1.0, np.float32)
        for b in range(NBLK):
            rr, dd, nev = seg[(k, b)]
            n = rr.shape[0]
            c0 = int(off_b[b])
            nch = int(counts_b[b])
            fl_i = idx[c0 : c0 + nch].reshape(-1)
            fl_d = dstv[c0 : c0 + nch].reshape(-1, 2)
            fl_i[:n] = rr.astype(np.int16)
            labv = np.repeat(labels[c0 : c0 + nch], P)[:n]
            par = (np.arange(n) >= nev).astype(np.int64)
            lane = np.where(labv == 2, par, 0)
            fl_d[np.arange(n), lane] = dd.astype(np.float32)
        per_core.append(dict(
            idx=_wrap_idxs(idx.reshape(-1)),
            dstv=np.ascontiguousarray(dstv.transpose(1, 2, 0)).astype(BF16),
            invdeg=np.tile(
                inv_deg[k * ROWS_PER : (k + 1) * ROWS_PER][None, :], (P, 1)
            ).astype(BF16),
        ))

    meta = dict(
        perm=perm, off_b=off_b, C=C,
        counts_key=tuple(int(v) for v in counts_b),
        labels=tuple(int(v) for v in labels),
    )
    return meta, per_core


def build_graph(nc, m):
    dt = mybir.dt
    alu = mybir.AluOpType
    act = mybir.ActivationFunctionType
    C = m["C"]
    off_b = m["off_b"]
    labels = m["labels"]

    xp_d = nc.dram_tensor("xp", [N_NODES // 2, 2 * D_IN], dt.float8e4,
                          kind="ExternalInput")
    xT_d = nc.dram_tensor("xT", [P, ROWS_PER], dt.bfloat16, kind="ExternalInput")
    idx_d = nc.dram_tensor("idx", [P, C * 8], dt.int16, kind="ExternalInput")
    dstv_d = nc.dram_tensor("dstv", [P, 2, C], dt.bfloat16,
                            kind="ExternalInput")
    invdeg_d = nc.dram_tensor("invdeg", [P, ROWS_PER], dt.bfloat16,
                              kind="ExternalInput")
    iota_d = nc.dram_tensor("iota", [P, P], dt.bfloat16, kind="ExternalInput")
    w1l_d = nc.dram_tensor("w1lT", [P, D_HID], dt.bfloat16, kind="ExternalInput")
    w1r_d = nc.dram_tensor("w1rT", [P, D_HID], dt.bfloat16, kind="ExternalInput")
    w2l_d = nc.dram_tensor("w2lT", [P, D_OUT], dt.bfloat16, kind="ExternalInput")
    w2r_d = nc.dram_tensor("w2rT", [P, D_OUT], dt.bfloat16, kind="ExternalInput")
    b1_d = nc.dram_tensor("b1r", [1, D_HID], dt.bfloat16, kind="ExternalInput")
    b2_d = nc.dram_tensor("b2r", [1, D_OUT], dt.bfloat16, kind="ExternalInput")
    out_d = nc.dram_tensor("out", [ROWS_PER, D_OUT], dt.float32,
                           kind="ExternalOutput")

    with tile.TileContext(nc) as tc, ExitStack() as ctx:
        sb = ctx.enter_context(tc.tile_pool(name="sb", bufs=1))
        dram = ctx.enter_context(tc.tile_pool(name="dram", bufs=1, space="DRAM"))
        psA = ctx.enter_context(tc.tile_pool(name="psA", bufs=1, space="PSUM"))
        psB = ctx.enter_context(tc.tile_pool(name="psB", bufs=1, space="PSUM"))
        g_p = ctx.enter_context(tc.tile_pool(name="gp", bufs=GBUFS))
        o_p = ctx.enter_context(tc.tile_pool(name="oh", bufs=OBUFS))
        st_p = ctx.enter_context(tc.tile_pool(name="st", bufs=3))

        def load(shape, dtype, src, name):
            t = sb.tile(shape, dtype, name=name)
            nc.sync.dma_start(t[:], src[:])
            return t

        # small tiles first (they gate the first one-hot / matmul)
        iota_sb = load([P, P], dt.bfloat16, iota_d.ap(), "iota_sb")
        w1l_sb = load([P, D_HID], dt.bfloat16, w1l_d.ap(), "w1l_sb")
        w1r_sb = load([P, D_HID], dt.bfloat16, w1r_d.ap(), "w1r_sb")
        w2l_sb = load([P, D_OUT], dt.bfloat16, w2l_d.ap(), "w2l_sb")
        w2r_sb = load([P, D_OUT], dt.bfloat16, w2r_d.ap(), "w2r_sb")
        b1_sb = load([1, D_HID], dt.bfloat16, b1_d.ap(), "b1_sb")
        b2_sb = load([1, D_OUT], dt.bfloat16, b2_d.ap(), "b2_sb")

        # idx/dstv: load the first couple of gather groups' worth first,
        # then invdeg/xT (gating early drains), then the remainder
        idx_sb = sb.tile([P, C * 8], dt.int16, name="idx_sb")
        dstv_sb = sb.tile([P, 2, C], dt.bfloat16, name="dstv_sb")
        pieces = [(0, 2 * GRP), (2 * GRP, 8 * GRP), (8 * GRP, C)]
        a, b_ = pieces[0]
        nc.sync.dma_start(idx_sb[:, a * 8 : b_ * 8], idx_d.ap()[:, a * 8 : b_ * 8])
        nc.sync.dma_start(dstv_sb[:, :, a:b_], dstv_d.ap()[:, :, a:b_])
        invdeg_sb = load([P, ROWS_PER], dt.bfloat16, invdeg_d.ap(), "invdeg_sb")
        xT_sb = load([P, ROWS_PER], dt.bfloat16, xT_d.ap(), "xT_sb")
        for a, b_ in pieces[1:]:
            a = min(a, C); b_ = min(b_, C)
            if a >= b_:
                continue
            nc.sync.dma_start(idx_sb[:, a * 8 : b_ * 8],
                              idx_d.ap()[:, a * 8 : b_ * 8])
            nc.sync.dma_start(dstv_sb[:, :, a:b_], dstv_d.ap()[:, :, a:b_])

        ones_sb = sb.tile([1, 512], dt.bfloat16, name="ones_sb")
        nc.vector.memset(ones_sb[:], 1.0)

        meanT = sb.tile([P, ROWS_PER], dt.bfloat16, name="meanT")
        meanhT = sb.tile([P, ROWS_PER], dt.bfloat16, name="meanhT")
        hT = sb.tile([P, ROWS_PER], dt.bfloat16, name="hT")

        hsh = dram.tile([ROWS_PER, D_IN], dt.float8e4, name="hsh")
        hfull = dram.tile([N_NODES // 2, 2 * D_IN], dt.float8e4, name="hfull")

        qctr = [0]
        src_ap = [xp_d.ap(), hfull[:]]
        tiles = {}

        def ensure_group(layer, g):
            if (layer, g) in tiles:
                return tiles[(layer, g)]
            c0, c1 = g * GRP, min(C, (g + 1) * GRP)
            nch = c1 - c0
            n = nch * P
            t = g_p.tile([P, GRP, 2 * D_IN], dt.float8e4, tag="gt", name="gt")
            nc.gpsimd.dma_gather(
                t[:, :nch, :], src_ap[layer],
                idx_sb[:, c0 * 8 : c1 * 8],
                n, n, 2 * D_IN, elem_step=2 * D_IN, single_packet=False,
                queue_num=qctr[0] % NQ,
            )
            qctr[0] += 1
            ot = o_p.tile([P, 2, GRP, BW], dt.bfloat16, tag="ohv", name="ohv")
            # lane 0 serves every chunk; lane 1 only mixed runs
            for h0 in range(0, nch, GRP // 2):
                h1 = min(nch, h0 + GRP // 2)
                nc.vector.tensor_tensor(
                    ot[:, 0, h0:h1, :],
                    iota_sb[:, None, :BW].broadcast_to([P, h1 - h0, BW]),
                    dstv_sb[:, 0, c0 + h0 : c0 + h1, None].broadcast_to(
                        [P, h1 - h0, BW]),
                    alu.is_equal,
                )
            h0 = 0
            while h0 < nch:
                if labels[c0 + h0] != 2:
                    h0 += 1
                    continue
                h1 = h0 + 1
                while h1 < nch and labels[c0 + h1] == 2:
                    h1 += 1
                nc.vector.tensor_tensor(
                    ot[:, 1, h0:h1, :],
                    iota_sb[:, None, :BW].broadcast_to([P, h1 - h0, BW]),
                    dstv_sb[:, 1, c0 + h0 : c0 + h1, None].broadcast_to(
                        [P, h1 - h0, BW]),
                    alu.is_equal,
                )
                h0 = h1
            tiles[(layer, g)] = (t, ot)
            return tiles[(layer, g)]

        def accum_block(layer, b, psum):
            cs, ce = int(off_b[b]), int(off_b[b + 1])
            nmm = sum(2 if labels[c] == 2 else 1 for c in range(cs, ce))
            i = 0
            for c in range(cs, ce):
                gt, ot = ensure_group(layer, c // GRP)
                j = c % GRP
                lab = labels[c]
                parities = (0, 1) if lab == 2 else (lab,)
                for o in parities:
                    lane = o if lab == 2 else 0
                    nc.tensor.matmul(
                        psum[:, :BW],
                        lhsT=gt[:, j, o * D_IN : (o + 1) * D_IN],
                        rhs=ot[:, lane, j, :],
                        start=(i == 0), stop=(i == nmm - 1),
                    )
                    i += 1
            return nmm

        def h_rows(bp):
            """h row computation for 128-column block pair bp."""
            c0 = bp * P
            bs = min(P, ROWS_PER - c0)
            ps2 = psB.tile([P, 512], dt.float32, tag="ps", name="ps_r", bufs=3)
            nc.tensor.matmul(ps2[:bs, :D_HID], lhsT=meanT[:, c0 : c0 + bs],
                             rhs=w1l_sb[:], start=True, stop=False)
            nc.tensor.matmul(ps2[:bs, :D_HID], lhsT=xT_sb[:, c0 : c0 + bs],
                             rhs=w1r_sb[:], start=False, stop=False)
            nc.tensor.matmul(ps2[:bs, :D_HID], lhsT=ones_sb[:, :bs],
                             rhs=b1_sb[:], start=False, stop=True)
            hrow = st_p.tile([P, D_HID], dt.float8e4, tag="st", name="hrow")
            nc.scalar.activation(hrow[:bs, :], ps2[:bs, :D_HID], act.Relu)
            nc.sync.dma_start(hsh[c0 : c0 + bs, :], hrow[:bs, :])

        # ================= layer 1 =================
        LAG = 6
        for b in range(NBLK):
            c0 = b * BW
            bs = min(BW, ROWS_PER - c0)
            ps = psA.tile([P, BW], dt.float32, tag="agg", name="ps_agg", bufs=5)
            accum_block(0, b, ps)
            nc.vector.tensor_tensor(
                meanT[:, c0 : c0 + bs], ps[:, :bs],
                invdeg_sb[:, c0 : c0 + bs], alu.mult,
            )
            if b % 2 == 1 and b // 2 >= LAG:
                h_rows(b // 2 - LAG)
        for bp in range((NBLK + 1) // 2 - LAG, (NBLK + 1) // 2):
            h_rows(bp)

        nc.gpsimd.collective_compute(
            "AllGather", alu.bypass,
            replica_groups=[list(range(N_CORES))],
            ins=[hsh[:].opt()], outs=[hfull[:].opt()],
        )

        # col-major bf16 h panels (dense path of layer 2)
        for c0 in range(0, ROWS_PER, 512):
            w = min(512, ROWS_PER - c0)
            ps2 = psB.tile([P, 512], dt.float32, tag="ps", name="ps_d", bufs=3)
            nc.tensor.matmul(ps2[:, :w], lhsT=w1l_sb[:], rhs=meanT[:, c0 : c0 + w],
                             start=True, stop=False)
            nc.tensor.matmul(ps2[:, :w], lhsT=w1r_sb[:], rhs=xT_sb[:, c0 : c0 + w],
                             start=False, stop=False)
            nc.tensor.matmul(ps2[:, :w], lhsT=b1_sb[:], rhs=ones_sb[:, :w],
                             start=False, stop=True)
            nc.scalar.activation(hT[:, c0 : c0 + w], ps2[:, :w], act.Relu)

        # ================= layer 2 =================
        def emit_out(bp):
            p0 = bp * P
            pbs = min(P, ROWS_PER - p0)
            ps2 = psB.tile([P, 512], dt.float32, tag="ps", name="ps_o", bufs=3)
            nc.tensor.matmul(ps2[:pbs, :D_OUT], lhsT=meanhT[:, p0 : p0 + pbs],
                             rhs=w2l_sb[:], start=True, stop=False)
            nc.tensor.matmul(ps2[:pbs, :D_OUT], lhsT=hT[:, p0 : p0 + pbs],
                             rhs=w2r_sb[:], start=False, stop=False)
            nc.tensor.matmul(ps2[:pbs, :D_OUT], lhsT=ones_sb[:, :pbs],
                             rhs=b2_sb[:], start=False, stop=True)
            otile = st_p.tile([P, D_OUT], dt.float32, tag="ot", name="otile")
            nc.vector.tensor_copy(otile[:pbs, :], ps2[:pbs, :D_OUT])
            nc.sync.dma_start(out_d.ap()[p0 : p0 + pbs, :], otile[:pbs, :])

        for b in range(NBLK):
            c0 = b * BW
            bs = min(BW, ROWS_PER - c0)
            ps = psA.tile([P, BW], dt.float32, tag="agg", name="ps_l2", bufs=5)
            accum_block(1, b, ps)
            nc.vector.tensor_tensor(meanhT[:, c0 : c0 + bs], ps[:, :bs],
                                    invdeg_sb[:, c0 : c0 + bs], alu.mult)
            if b % 2 == 1 and b // 2 >= LAG:
                emit_out(b // 2 - LAG)
        for bp in range((NBLK + 1) // 2 - LAG, (NBLK + 1) // 2):
            emit_out(bp)

    return nc


def make_in_maps(inputs, meta, per_core):
    x = np.asarray(inputs["x"], np.float32)[meta["perm"]]
    xp = x.astype(FP8).reshape(N_NODES // 2, 2 * D_IN)
    w1l = np.asarray(inputs["W1l"], np.float32)
    w1r = np.asarray(inputs["W1r"], np.float32)
    w2l = np.asarray(inputs["W2l"], np.float32)
    w2r = np.asarray(inputs["W2r"], np.float32)
    b1 = np.asarray(inputs["b1"], np.float32)
    b2 = np.asarray(inputs["b2"], np.float32)
    iota = np.tile(np.arange(P, dtype=np.float32)[None, :], (P, 1)).astype(BF16)
    in_maps = []
    for k in range(N_CORES):
        pc = per_core[k]
        im = {
            "xp": xp,
            "xT": np.ascontiguousarray(
                x[k * ROWS_PER : (k + 1) * ROWS_PER].T).astype(BF16),
            "invdeg": pc["invdeg"],
            "iota": iota,
            "w1lT": np.ascontiguousarray(w1l.T).astype(BF16),
            "w1rT": np.ascontiguousarray(w1r.T).astype(BF16),
            "w2lT": np.ascontiguousarray(w2l.T).astype(BF16),
            "w2rT": np.ascontiguousarray(w2r.T).astype(BF16),
            "b1r": b1[None, :].astype(BF16),
            "b2r": b2[None, :].astype(BF16),
        }
        im["idx"] = pc["idx"]
        im["dstv"] = pc["dstv"]
        in_maps.append(im)
    return in_maps


_CACHE = {}


def _compile(meta):
    key = (meta["counts_key"], meta["labels"])
    if key not in _CACHE:
        nc = bacc.Bacc("TRN2", target_bir_lowering=False, debug=False,
                       num_devices=N_CORES, num_swdge_queues=NQ)
        build_graph(nc, meta)
        nc.compile()
        _CACHE[key] = nc
    return _CACHE[key]


def assemble(res, meta):
    out = np.concatenate(
        [np.asarray(res.results[k]["out"]) for k in range(N_CORES)], axis=0
    ).astype(np.float32)
    unperm = np.empty_like(out)
    unperm[meta["perm"]] = out
    return unperm


def kernel(**inputs):
    edge_index = np.asarray(inputs["edge_index"])
    meta, per_core = preprocess(edge_index)
    nc = _compile(meta)
    in_maps = make_in_maps(inputs, meta, per_core)
    res = bass_utils.run_bass_kernel_spmd(
        nc, in_maps, core_ids=list(range(N_CORES))
    )
    return assemble(res, meta)
